# revision 1
# baseline (speedup 1.0000x reference)
"""DiGCN link prediction on 8 TRN2 NeuronCores.

Math (reference):
    h1 = relu(segsum_dst(w_e * (x@W1)[src]) + b1)
    h2 = segsum_dst(w_e * (h1@W2)[src]) + b2
    logits = concat(h2[qs], h2[qd]) @ Wl + bl ; out = log_softmax(logits)

Device strategy (per core, SPMD-identical graph, per-core data):
  - dst-sharded edges. Host packs each core's dsts into fixed 16-column
    windows (whole dsts, FFD), 32 windows per 512-col PSUM group.
  - Layer 1 uses linearity: segsum(w, x@W1) = segsum(w, x)@W1. Per window
    two 128-slot gather blocks (src<25000 and src>=25000 halves, int16
    dma_gather from the two x table halves), per-block matmul
    msgs^T[128e,128f] @ S[128e,16] into feature-major PSUM; the hi pass
    adds on top in SBUF. Then project W1^T (f32r) + relu + b1 -> h1T.
  - Layer 2 + head use linearity again: with A=W2@Wl[:256], B=W2@Wl[256:],
    u[d]=sum w_e*(h1@A)[src]+b2@Wlt, v[d]=sum w_e*(h1@B)[src]+b2@Wlb+bl,
    logits[q] = u[qs]+v[qd]. yab=h1@[A|B] ([*,4] bf16) is AllGathered
    (1 MB), padded locally into a 256B-row table, aggregated with the same
    window structure (3 sub-blocks per window, split by src owner-core
    group for int16).
  - Query head: queries are processed where qs lives (local u gather from
    a padded 256B-row u table), the u-halves are AllToAll'd to the qd
    owner, which gathers v locally, adds, and takes log_softmax.
"""

import math
from contextlib import ExitStack

import ml_dtypes
import numpy as np

import concourse.bass as bass
import concourse.tile as tile
from concourse import bacc, mybir
from concourse.masks import make_identity

BF16 = mybir.dt.bfloat16
F32 = mybir.dt.float32
F32R = mybir.dt.float32r
I16 = mybir.dt.int16
P = 128
WCOLS = 16      # columns per window
GW = 32         # windows per PSUM group (512 cols)


def _wrap_idx(stream):
    """[n] int -> [128, n//16] int16 in dma_gather wrapped layout."""
    n = len(stream)
    a = np.asarray(stream, np.int64).reshape(n // 16, 16).T
    return np.tile(a, (8, 1)).astype(np.int16)


def plan(inputs, n_cores=8, verbose=False):
    x = np.asarray(inputs["x"], np.float32)
    edge_index = np.asarray(inputs["edge_index"], np.int64)
    query_edges = np.asarray(inputs["query_edges"], np.int64)
    edge_weight = np.asarray(inputs["edge_weight"], np.float32)
    W1 = np.asarray(inputs["W1"], np.float32)
    b1 = np.asarray(inputs["b1"], np.float32)
    W2 = np.asarray(inputs["W2"], np.float32)
    b2 = np.asarray(inputs["b2"], np.float32)
    Wl = np.asarray(inputs["Wl"], np.float32)
    bl = np.asarray(inputs["bl"], np.float32)

    N, F = x.shape
    E = edge_index.shape[1]
    Q = query_edges.shape[0]
    assert F == 256 and N % n_cores == 0
    n_local = N // n_cores
    NH = N // 2                      # x table split point
    cg = [0, 3 * n_local * 3 // 3, 0, 0]  # placeholder
    # owner-core groups for the L2 table third-split: {0,1,2},{3,4,5},{6,7}
    g_of_core = np.array([0, 0, 0, 1, 1, 1, 2, 2][:n_cores])
    tb = [np.searchsorted(g_of_core, t) * n_local for t in range(3)]
    tb.append(N)  # third t covers nodes [tb[t], tb[t+1])

    src = edge_index[0]
    dst = edge_index[1]
    qs, qd = query_edges[:, 0], query_edges[:, 1]

    # ---- pack windows per core (FFD by degree desc, 6 caps) ----
    CAP = 126
    packs = []
    W = 0
    for c in range(n_cores):
        m = dst // n_local == c
        ed = dst[m] - c * n_local
        es = src[m]
        deg = np.bincount(ed, minlength=n_local)
        lo1 = np.bincount(ed[es < NH], minlength=n_local)
        t0 = np.bincount(ed[es < tb[1]], minlength=n_local)
        t1 = np.bincount(ed[(es >= tb[1]) & (es < tb[2])], minlength=n_local)
        hi1 = deg - lo1
        t2 = deg - t0 - t1
        assert max(lo1.max(), hi1.max(), t0.max(), t1.max(), t2.max()) <= CAP
        order = np.argsort(-deg, kind="stable")
        caps = np.stack([lo1, hi1, t0, t1, t2], 1)  # [n_local, 5]
        win_of = np.empty(n_local, np.int64)
        rank_of = np.empty(n_local, np.int64)
        wins_used = []   # list of [5] counts
        wins_n = []      # dsts per window
        SCAN = 24
        for d in order:
            cd = caps[d]
            placed = False
            for wi in range(len(wins_used) - 1, max(-1, len(wins_used) - 1 - SCAN), -1):
                if wins_n[wi] < WCOLS and np.all(wins_used[wi] + cd <= CAP):
                    win_of[d] = wi
                    rank_of[d] = wins_n[wi]
                    wins_used[wi] += cd
                    wins_n[wi] += 1
                    placed = True
                    break
            if not placed:
                win_of[d] = len(wins_used)
                rank_of[d] = 0
                wins_used.append(cd.copy())
                wins_n.append(1)
        packs.append((m, win_of, rank_of))
        W = max(W, len(wins_used))
    W = ((W + GW - 1) // GW) * GW
    COLS = WCOLS * W
    NT = COLS // P
    assert 3 * COLS < 2 ** 15, "L2 table third exceeds int16 range"
    n_grp = W // GW

    # column & global row of every node
    col_all = np.empty(N, np.int64)
    g_row = np.empty(N, np.int64)
    for c in range(n_cores):
        m, win_of, rank_of = packs[c]
        col = win_of * WCOLS + rank_of
        col_all[c * n_local:(c + 1) * n_local] = col
        g_row[c * n_local:(c + 1) * n_local] = \
            c * COLS + (col % P) * NT + col // P

    # ---- per-core edge streams ----
    i1_l, s1_l, i2_l, s2_l = [], [], [], []
    for c in range(n_cores):
        m, win_of, rank_of = packs[c]
        es, ew = src[m], edge_weight[m]
        ed = dst[m] - c * n_local
        ewin = win_of[ed]
        erank = rank_of[ed]
        half1 = (es >= NH).astype(np.int64)
        third = np.searchsorted(np.array(tb[1:3]), es, side="right")

        def build(nsub, sub, base_vals, n_blocks_per_grp):
            # block of edge = grp*(nsub*GW) + sub*GW + (win % GW)
            grp = ewin // GW
            blk = grp * (nsub * GW) + sub * GW + (ewin % GW)
            nblk = n_grp * nsub * GW
            # slot within block: stable order by (blk), cumcount
            order_e = np.lexsort((np.arange(len(es)), blk))
            bsort = blk[order_e]
            first = np.concatenate([[True], bsort[1:] != bsort[:-1]])
            start_pos = np.maximum.accumulate(
                np.where(first, np.arange(len(es)), 0))
            slot_sorted = np.arange(len(es)) - start_pos
            slot = np.empty(len(es), np.int64)
            slot[order_e] = slot_sorted
            assert slot.max(initial=0) < P
            idx = np.zeros((nblk, P), np.int64)
            S = np.zeros((nblk, P, WCOLS), np.float32)
            idx[blk, slot] = base_vals
            S[blk, slot, erank] = ew
            return idx, S

        sub1 = half1
        base1 = np.where(es < NH, es, es - NH)
        idx1, S1 = build(2, sub1, base1, 2 * GW)
        base2 = g_row[es] - np.array([0, tb[1] // n_local * COLS,
                                      tb[2] // n_local * COLS])[third]
        idx2, S2 = build(3, third, base2, 3 * GW)

        # wrapped per-call idx [ncalls, 128, GW*P/16], S [ncalls, 128, GW, 16]
        def to_calls(idx, S, nsub):
            ncalls = n_grp * nsub
            iw = np.empty((ncalls, P, GW * P // 16), np.int16)
            sw = np.empty((ncalls, P, GW, WCOLS), ml_dtypes.bfloat16)
            for call in range(ncalls):
                blocks = idx[call * GW:(call + 1) * GW]      # [GW, P]
                stream = blocks.reshape(GW * P)              # pos j*128+p -> block j slot p
                iw[call] = _wrap_idx(stream)
                sblk = S[call * GW:(call + 1) * GW]          # [GW, P, 16]
                sw[call] = sblk.transpose(1, 0, 2).astype(ml_dtypes.bfloat16)
            return iw, sw

        iw1, sw1 = to_calls(idx1, S1, 2)
        iw2, sw2 = to_calls(idx2, S2, 3)
        i1_l.append(iw1)
        s1_l.append(sw1)
        i2_l.append(iw2)
        s2_l.append(sw2)

    # ---- queries: gather u at owner(qs), AllToAll to owner(qd) ----
    q_owner_s = qs // n_local
    q_owner_d = qd // n_local
    counts = np.zeros((n_cores, n_cores), np.int64)
    np.add.at(counts, (q_owner_s, q_owner_d), 1)
    QSLOT = ((int(counts.max()) + P - 1) // P) * P
    QTOT = n_cores * QSLOT
    QJ = QTOT // P
    loc_row = (col_all % P) * NT + col_all // P  # local table row of node

    qu_l, qv_l = [], []
    send_pos = np.empty(Q, np.int64)   # position in sender's stream
    for c in range(n_cores):
        mine = np.nonzero(q_owner_s == c)[0]
        dests = q_owner_d[mine]
        order = np.argsort(dests, kind="stable")
        mine = mine[order]
        dests = dests[order]
        qu = np.zeros(QTOT, np.int64)
        fill = np.zeros(n_cores, np.int64)
        pos = np.empty(len(mine), np.int64)
        for ii, (q, d) in enumerate(zip(mine, dests)):
            pos[ii] = d * QSLOT + fill[d]
            fill[d] += 1
        qu[pos] = loc_row[qs[mine]]
        send_pos[mine] = pos
        qu_l.append(_wrap_idx(qu))
    # receiver side: position in a2a_out = s*QSLOT + slot
    qv_l = []
    out_map = []  # per core: array [QTOT] of orig query index or -1
    for c in range(n_cores):
        qv = np.zeros(QTOT, np.int64)
        omap = np.full(QTOT, -1, np.int64)
        for s in range(n_cores):
            sel = np.nonzero((q_owner_s == s) & (q_owner_d == c))[0]
            # slots assigned in sender order
            slots = send_pos[sel] - c * QSLOT  # slot within bucket
            qv[s * QSLOT + slots] = loc_row[qd[sel]]
            omap[s * QSLOT + slots] = sel
        qv_l.append(_wrap_idx(qv))
        out_map.append(omap)

    # ---- weights / constants ----
    AB = np.concatenate([W2 @ Wl[:256], W2 @ Wl[256:]], axis=1)  # [256,4]
    cu = b2 @ Wl[:256]
    cv = b2 @ Wl[256:] + bl
    cuv = np.concatenate([cu, cv]).reshape(4, 1).astype(np.float32)
    b1c = b1.reshape(2, P).astype(np.float32)
    x_bf = x.astype(ml_dtypes.bfloat16)
    w1_f = np.ascontiguousarray(W1.astype(np.float32))
    ab_f = np.ascontiguousarray(AB.astype(np.float32))

    in_maps = []
    for c in range(n_cores):
        in_maps.append({
            "x": x_bf, "i1": i1_l[c], "s1": s1_l[c],
            "i2": i2_l[c], "s2": s2_l[c],
            "qu": qu_l[c], "qv": qv_l[c],
            "w1": w1_f, "ab": ab_f, "b1": b1c, "cuv": cuv,
        })

    dims = dict(N=N, NH=NH, W=W, COLS=COLS, NT=NT, QJ=QJ, QSLOT=QSLOT,
                n_grp=n_grp, n_cores=n_cores,
                tsplit=(tb[1] // n_local, tb[2] // n_local))
    if verbose:
        fill1 = E / (n_cores * n_grp * 2 * GW * P)
        fill2 = E / (n_cores * n_grp * 3 * GW * P)
        print(f"plan: W={W} COLS={COLS} NT={NT} QSLOT={QSLOT} QJ={QJ} "
              f"fill1={fill1:.3f} fill2={fill2:.3f}")
    meta = dict(out_map=out_map, Q=Q, QJ=QJ)
    return dims, in_maps, meta


def unshard(results, meta):
    Q, QJ = meta["Q"], meta["QJ"]
    out = np.empty((Q, 2), np.float32)
    for c, res in enumerate(results):
        o = res["out"].reshape(P * QJ, 2)
        omap = meta["out_map"][c]
        # out rows: position pi lives at (p=pi%128, j=pi//128) -> flat p*QJ+j
        pi = np.nonzero(omap >= 0)[0]
        out[omap[pi]] = o[(pi % P) * QJ + pi // P]
    return out


# ----------------------------------------------------------------------------
# Device graph
# ----------------------------------------------------------------------------

def build_nc(dims):
    n_cores = dims["n_cores"]
    N, NH, COLS, NT, QJ = (dims["N"], dims["NH"], dims["COLS"], dims["NT"],
                           dims["QJ"])
    n_grp = dims["n_grp"]
    QTOT = QJ * P

    nc = bacc.Bacc("TRN2", target_bir_lowering=False, debug=False,
                   enable_asserts=False, num_devices=n_cores,
                   num_swdge_queues=4)

    IW = GW * P // 16
    t_x = nc.dram_tensor("x", [N, 256], BF16, kind="ExternalInput")
    t_i1 = nc.dram_tensor("i1", [n_grp * 2, P, IW], I16, kind="ExternalInput")
    t_s1 = nc.dram_tensor("s1", [n_grp * 2, P, GW, WCOLS], BF16,
                          kind="ExternalInput")
    t_i2 = nc.dram_tensor("i2", [n_grp * 3, P, IW], I16, kind="ExternalInput")
    t_s2 = nc.dram_tensor("s2", [n_grp * 3, P, GW, WCOLS], BF16,
                          kind="ExternalInput")
    t_qu = nc.dram_tensor("qu", [P, QTOT // 16], I16, kind="ExternalInput")
    t_qv = nc.dram_tensor("qv", [P, QTOT // 16], I16, kind="ExternalInput")
    t_w1 = nc.dram_tensor("w1", [256, 256], F32, kind="ExternalInput")
    t_ab = nc.dram_tensor("ab", [256, 4], F32, kind="ExternalInput")
    t_b1 = nc.dram_tensor("b1", [2, P], F32, kind="ExternalInput")
    t_cuv = nc.dram_tensor("cuv", [4, 1], F32, kind="ExternalInput")
    t_out = nc.dram_tensor("out", [P, QJ, 2], F32, kind="ExternalOutput")

    t_yab = nc.dram_tensor("yab_l", [P, NT * 4], BF16)
    t_uvc = nc.dram_tensor("uvc", [n_cores * P, NT * 4], BF16,
                           addr_space="Shared")
    c0, c1 = dims["tsplit"]
    t_uvp0 = nc.dram_tensor("uvp0", [c0 * COLS, P], BF16)
    t_uvp1 = nc.dram_tensor("uvp1", [(c1 - c0) * COLS, P], BF16)
    t_uvp2 = nc.dram_tensor("uvp2", [(n_cores - c1) * COLS, P], BF16)
    t_upad = nc.dram_tensor("upad", [COLS, 64], F32)
    t_vpad = nc.dram_tensor("vpad", [COLS, 64], F32)
    t_a2i = nc.dram_tensor("a2i", [QTOT, 2], F32)
    t_a2o = nc.dram_tensor("a2o", [QTOT, 2], F32)

    tensors = locals()
    with tile.TileContext(nc) as tc:
        with ExitStack() as ctx:
            _emit(ctx, tc, dims, tensors)
    nc.compile()
    return nc


def _emit(ctx, tc, dims, T):
    nc = tc.nc
    n_cores = dims["n_cores"]
    N, NH, COLS, NT, QJ = (dims["N"], dims["NH"], dims["COLS"], dims["NT"],
                           dims["QJ"])
    n_grp = dims["n_grp"]
    c0, c1 = dims["tsplit"]
    QTOT = QJ * P
    IW = GW * P // 16
    NI = GW * P
    Relu = mybir.ActivationFunctionType.Relu
    Copy = mybir.ActivationFunctionType.Copy
    Exp = mybir.ActivationFunctionType.Exp
    Ln = mybir.ActivationFunctionType.Ln

    const = ctx.enter_context(tc.tile_pool(name="const", bufs=1))

    w1A = const.tile([P, 256], F32)
    nc.sync.dma_start(w1A[:], T["t_w1"].ap()[0:P, :])
    w1B = const.tile([P, 256], F32)
    nc.sync.dma_start(w1B[:], T["t_w1"].ap()[P:256, :])
    w1Ar = const.tile([P, 256], F32R)
    nc.vector.tensor_copy(w1Ar[:], w1A[:])
    w1Br = const.tile([P, 256], F32R)
    nc.vector.tensor_copy(w1Br[:], w1B[:])
    abA = const.tile([P, 4], F32)
    nc.sync.dma_start(abA[:], T["t_ab"].ap()[0:P, :])
    abB = const.tile([P, 4], F32)
    nc.sync.dma_start(abB[:], T["t_ab"].ap()[P:256, :])
    b1A = const.tile([P, 1], F32)
    nc.sync.dma_start(b1A[:], T["t_b1"].ap()[0, :, None])
    b1B = const.tile([P, 1], F32)
    nc.sync.dma_start(b1B[:], T["t_b1"].ap()[1, :, None])
    cuv = const.tile([4, 1], F32)
    nc.sync.dma_start(cuv[:], T["t_cuv"].ap()[:, :])
    qu = const.tile([P, QTOT // 16], I16)
    nc.sync.dma_start(qu[:], T["t_qu"].ap()[:, :])
    qv = const.tile([P, QTOT // 16], I16)
    nc.sync.dma_start(qv[:], T["t_qv"].ap()[:, :])
    id4 = const.tile([4, 4], F32)
    make_identity(nc, id4[:])

    # long-lived tail tiles (before h1p: pool closes stay LIFO)
    tail = ctx.enter_context(tc.tile_pool(name="tail", bufs=1))
    ystage = tail.tile([P, NT * 4], BF16)
    uvT = tail.tile([4, COLS], F32)
    uvn = tail.tile([P, NT, 4], F32)

    h1pool_cm = tc.tile_pool(name="h1p", bufs=1)
    h1pool = h1pool_cm.__enter__()
    h1A = h1pool.tile([P, COLS], F32)
    h1B = h1pool.tile([P, COLS], F32)

    x_views = [T["t_x"].ap()[0:NH, :], T["t_x"].ap()[NH:N, :]]

    # ---------------- layer 1 ----------------
    with tc.tile_pool(name="msgs", bufs=3) as msgs_pool, \
         tc.tile_pool(name="idxp", bufs=3) as idxp, \
         tc.tile_pool(name="sp", bufs=3) as sp, \
         tc.tile_pool(name="aggp", bufs=3) as aggp, \
         tc.tile_pool(name="ps1", bufs=2, space="PSUM") as ps1, \
         tc.tile_pool(name="ps1b", bufs=2, space="PSUM") as ps1b, \
         tc.tile_pool(name="psz", bufs=2, space="PSUM") as psz:
        for g in range(n_grp):
            agA = aggp.tile([P, GW * WCOLS], F32R, tag="agA")
            agB = aggp.tile([P, GW * WCOLS], F32R, tag="agB")
            for half in range(2):
                call = g * 2 + half
                idxt = idxp.tile([P, IW], I16, tag="ix")
                nc.sync.dma_start(idxt[:], T["t_i1"].ap()[call, :, :])
                st = sp.tile([P, GW, WCOLS], BF16, tag="s")
                nc.sync.dma_start(st[:], T["t_s1"].ap()[call, :, :, :])
                mts = []
                for s in range(NI // 1024):
                    mt = msgs_pool.tile([P, 8, 256], BF16, tag=f"m1_{s}")
                    nc.gpsimd.dma_gather(
                        mt[:], x_views[half],
                        idxt[:, 64 * s:64 * (s + 1)], 1024, 1024, 256,
                        single_packet=False, queue_num=s)
                    mts.append(mt)
                pA = ps1.tile([P, GW * WCOLS], F32, tag="pA")
                pB = ps1b.tile([P, GW * WCOLS], F32, tag="pB")
                for j in range(GW):
                    cs = slice(WCOLS * j, WCOLS * (j + 1))
                    mt = mts[j // 8]
                    jj = j % 8
                    nc.tensor.matmul(pA[:, cs], lhsT=mt[:, jj, 0:P],
                                     rhs=st[:, j, :],
                                     start=(j == 0), stop=(j == GW - 1))
                    nc.tensor.matmul(pB[:, cs], lhsT=mt[:, jj, P:256],
                                     rhs=st[:, j, :],
                                     start=(j == 0), stop=(j == GW - 1))
                if half == 0:
                    nc.scalar.activation(agA[:], pA[:], Copy)
                    nc.vector.tensor_copy(agB[:], pB[:])
                else:
                    nc.vector.tensor_tensor(agA[:], agA[:], pA[:],
                                            op=mybir.AluOpType.add)
                    nc.vector.tensor_tensor(agB[:], agB[:], pB[:],
                                            op=mybir.AluOpType.add)
            cols = slice(g * GW * WCOLS, (g + 1) * GW * WCOLS)
            for m in range(2):
                pz = psz.tile([P, GW * WCOLS], F32, tag="pz")
                nc.tensor.matmul(pz[:], lhsT=w1Ar[:, m * P:(m + 1) * P],
                                 rhs=agA[:], start=True, stop=False)
                nc.tensor.matmul(pz[:], lhsT=w1Br[:, m * P:(m + 1) * P],
                                 rhs=agB[:], start=False, stop=True)
                h1m = h1A if m == 0 else h1B
                b1m = b1A if m == 0 else b1B
                nc.scalar.activation(h1m[:, cols], pz[:], Relu,
                                     bias=b1m[:, 0:1])
    # ---------------- yab = h1 @ [A|B] ----------------
    with tc.tile_pool(name="psyl", bufs=1, space="PSUM") as psyl:
        py = psyl.tile([P, NT * 4], F32)
        for t in range(NT):
            nc.tensor.matmul(py[:, 4 * t:4 * t + 4],
                             lhsT=h1A[:, t * P:(t + 1) * P], rhs=abA[:],
                             start=True, stop=False)
            nc.tensor.matmul(py[:, 4 * t:4 * t + 4],
                             lhsT=h1B[:, t * P:(t + 1) * P], rhs=abB[:],
                             start=False, stop=(True))
        nc.vector.tensor_copy(ystage[:], py[:])
    nc.sync.dma_start(T["t_yab"].ap()[:, :], ystage[:])
    h1pool_cm.__exit__(None, None, None)

    # ---------------- AllGather yab + pad-spray ----------------
    nc.gpsimd.collective_compute(
        "AllGather", mybir.AluOpType.bypass,
        replica_groups=[list(range(n_cores))],
        ins=[T["t_yab"].ap().opt()],
        outs=[T["t_uvc"].ap().opt()],
    )
    uvc_rows = T["t_uvc"].ap().rearrange("a (b c) -> (a b) c", c=4)
    third_starts = [0, c0 * COLS, c1 * COLS, n_cores * COLS]
    uvp_t = [T["t_uvp0"], T["t_uvp1"], T["t_uvp2"]]
    spray_eng = [nc.sync, nc.scalar, nc.sync]
    for t in range(3):
        spray_eng[t].dma_start(
            uvp_t[t].ap()[:, 0:4],
            uvc_rows[third_starts[t]:third_starts[t + 1], :])
    tuv_views = [uvp_t[t].ap()[:, :] for t in range(3)]

    # ---------------- layer 2 ----------------
    with tc.tile_pool(name="m2", bufs=4) as m2pool, \
         tc.tile_pool(name="idxp2", bufs=4) as idxp2, \
         tc.tile_pool(name="sp2", bufs=4) as sp2, \
         tc.tile_pool(name="ps2", bufs=2, space="PSUM") as ps2:
        for g in range(n_grp):
            puv = ps2.tile([4, GW * WCOLS], F32, tag="puv")
            for third in range(3):
                call = g * 3 + third
                idxt = idxp2.tile([P, IW], I16, tag="ix2")
                nc.sync.dma_start(idxt[:], T["t_i2"].ap()[call, :, :])
                st = sp2.tile([P, GW, WCOLS], BF16, tag="s2")
                nc.sync.dma_start(st[:], T["t_s2"].ap()[call, :, :, :])
                mt2s = []
                for s in range(NI // 1024):
                    mt2 = m2pool.tile([P, 8, P], BF16, tag=f"m2_{s}")
                    nc.gpsimd.dma_gather(
                        mt2[:], tuv_views[third],
                        idxt[:, 64 * s:64 * (s + 1)], 1024, 1024, P,
                        single_packet=False, queue_num=s)
                    mt2s.append(mt2)
                for j in range(GW):
                    cs = slice(WCOLS * j, WCOLS * (j + 1))
                    nc.tensor.matmul(puv[:, cs], lhsT=mt2s[j // 8][:, j % 8, 0:4],
                                     rhs=st[:, j, :],
                                     start=(third == 0 and j == 0),
                                     stop=(third == 2 and j == GW - 1))
            nc.vector.tensor_tensor(
                uvT[:, g * GW * WCOLS:(g + 1) * GW * WCOLS], puv[:],
                cuv[:, 0:1].to_broadcast([4, GW * WCOLS]),
                op=mybir.AluOpType.add)

    # ---------------- transpose uvT -> node-major, build u/v tables -------
    with tc.tile_pool(name="pst", bufs=2, space="PSUM") as pst:
        for t in range(NT):
            ptp = pst.tile([P, 4], F32, tag="ptp")
            nc.tensor.transpose(ptp[:], uvT[:, t * P:(t + 1) * P], id4[:])
            nc.vector.tensor_copy(uvn[:, t, :], ptp[:])
    upad_rows = T["t_upad"].ap()[:, 0:2].rearrange("(p t) c -> p t c", p=P)
    vpad_rows = T["t_vpad"].ap()[:, 0:2].rearrange("(p t) c -> p t c", p=P)
    nc.sync.dma_start(upad_rows, uvn[:, :, 0:2])
    nc.sync.dma_start(vpad_rows, uvn[:, :, 2:4])

    # ---------------- query head ----------------
    qp = ctx.enter_context(tc.tile_pool(name="qp", bufs=1))
    ug = qp.tile([P, QJ, 64], F32)
    for s in range(QTOT // 1024):
        nc.gpsimd.dma_gather(
            ug[:, 8 * s:8 * (s + 1), :], T["t_upad"].ap()[:, :],
            qu[:, 64 * s:64 * (s + 1)], 1024, 1024, 64,
            single_packet=False, queue_num=s % 4)
    us = qp.tile([P, QJ, 2], F32)
    nc.vector.tensor_copy(us[:], ug[:, :, 0:2])
    a2i_v = T["t_a2i"].ap().rearrange("(j p) c -> p j c", p=P)
    nc.sync.dma_start(a2i_v, us[:])
    vg = qp.tile([P, QJ, 64], F32)
    for s in range(QTOT // 1024):
        nc.gpsimd.dma_gather(
            vg[:, 8 * s:8 * (s + 1), :], T["t_vpad"].ap()[:, :],
            qv[:, 64 * s:64 * (s + 1)], 1024, 1024, 64,
            single_packet=False, queue_num=s % 4)
    nc.gpsimd.collective_compute(
        "AllToAll", mybir.AluOpType.bypass,
        replica_groups=[list(range(n_cores))],
        ins=[T["t_a2i"].ap().opt()],
        outs=[T["t_a2o"].ap().opt()],
    )
    ut2 = qp.tile([P, QJ, 2], F32)
    a2o_v = T["t_a2o"].ap().rearrange("(j p) c -> p j c", p=P)
    nc.sync.dma_start(ut2[:], a2o_v)

    lg = qp.tile([P, QJ, 2], F32)
    nc.vector.tensor_tensor(lg[:], ut2[:], vg[:, :, 0:2],
                            op=mybir.AluOpType.add)
    mx = qp.tile([P, QJ, 1], F32)
    nc.vector.reduce_max(mx[:], lg[:], axis=mybir.AxisListType.X)
    tt = qp.tile([P, QJ, 2], F32)
    nc.vector.tensor_tensor(tt[:], lg[:], mx[:].to_broadcast([P, QJ, 2]),
                            op=mybir.AluOpType.subtract)
    ex = qp.tile([P, QJ, 2], F32)
    nc.scalar.activation(ex[:], tt[:], Exp)
    sm = qp.tile([P, QJ, 1], F32)
    nc.vector.reduce_sum(sm[:], ex[:], axis=mybir.AxisListType.X)
    ls = qp.tile([P, QJ, 1], F32)
    nc.scalar.activation(ls[:], sm[:], Ln)
    oo = qp.tile([P, QJ, 2], F32)
    nc.vector.tensor_tensor(oo[:], tt[:], ls[:].to_broadcast([P, QJ, 2]),
                            op=mybir.AluOpType.subtract)
    nc.sync.dma_start(T["t_out"].ap()[:, :, :], oo[:])


# ----------------------------------------------------------------------------
# numpy reference (mirrors reference.py math in f32)
# ----------------------------------------------------------------------------

def numpy_reference(inputs):
    x = np.asarray(inputs["x"], np.float32)
    ei = np.asarray(inputs["edge_index"], np.int64)
    qe = np.asarray(inputs["query_edges"], np.int64)
    w = np.asarray(inputs["edge_weight"], np.float32)
    W1, b1 = np.asarray(inputs["W1"], np.float32), np.asarray(inputs["b1"], np.float32)
    W2, b2 = np.asarray(inputs["W2"], np.float32), np.asarray(inputs["b2"], np.float32)
    Wl, bl = np.asarray(inputs["Wl"], np.float32), np.asarray(inputs["bl"], np.float32)
    N = x.shape[0]
    src, dst = ei[0], ei[1]

    def conv(h, W, b):
        z = h @ W
        msg = z[src] * w[:, None]
        agg = np.zeros((N, z.shape[1]), np.float32)
        np.add.at(agg, dst, msg)
        return agg + b

    h1 = np.maximum(conv(x, W1, b1), 0.0)
    h2 = conv(h1, W2, b2)
    q = np.concatenate([h2[qe[:, 0]], h2[qe[:, 1]]], axis=1)
    logits = q @ Wl + bl
    m = logits.max(axis=1, keepdims=True)
    e = np.exp(logits - m)
    return logits - m - np.log(e.sum(axis=1, keepdims=True))

# ----------------------------------------------------------------------------
# Entry point: full inputs in, full output out
# ----------------------------------------------------------------------------

LAST_RESULTS = None


def kernel(**inputs):
    """Takes the FULL (unsharded) inputs of nn_DiGCN_link_prediction and
    returns the full [N_QUERY, 2] float32 log-softmax output.

    Shards nodes/edges/queries across 8 NeuronCores internally, runs one
    SPMD Bass kernel (per-core data, identical graph), and reassembles.
    """
    global LAST_RESULTS
    import os
    from concourse.bass_utils import run_bass_kernel_spmd

    n_cores = 8
    dims, in_maps, meta = plan(inputs, n_cores=n_cores)
    nc = build_nc(dims)
    res = run_bass_kernel_spmd(
        nc, in_maps, core_ids=list(range(n_cores)),
        trace=bool(int(os.environ.get("GNN_TRACE", "0"))),
        stitch_traces=False,
    )
    LAST_RESULTS = res
    return unshard(res.results, meta)



# revision 14
# speedup vs baseline: 1.1950x; 1.1950x over previous
"""DiGCN link prediction on 8 TRN2 NeuronCores.

Math (reference):
    h1 = relu(segsum_dst(w_e * (x@W1)[src]) + b1)
    h2 = segsum_dst(w_e * (h1@W2)[src]) + b2
    logits = concat(h2[qs], h2[qd]) @ Wl + bl ; out = log_softmax(logits)

Device strategy (per core, SPMD-identical graph, per-core data):
  - dst-sharded edges. Host packs each core's dsts into fixed 16-column
    windows (whole dsts, FFD), 32 windows per 512-col PSUM group.
  - Layer 1 uses linearity: segsum(w, x@W1) = segsum(w, x)@W1. Per window
    two 128-slot gather blocks (src<25000 and src>=25000 halves, int16
    dma_gather from the two x table halves), per-block matmul
    msgs^T[128e,128f] @ S[128e,16] into feature-major PSUM; the hi pass
    adds on top in SBUF. Then project W1^T (f32r) + relu + b1 -> h1T.
  - Layer 2 + head use linearity again: with A=W2@Wl[:256], B=W2@Wl[256:],
    u[d]=sum w_e*(h1@A)[src]+b2@Wlt, v[d]=sum w_e*(h1@B)[src]+b2@Wlb+bl,
    logits[q] = u[qs]+v[qd]. yab=h1@[A|B] ([*,4] bf16) is AllGathered
    (1 MB), padded locally into a 256B-row table, aggregated with the same
    window structure (3 sub-blocks per window, split by src owner-core
    group for int16).
  - Query head: queries are processed where qs lives (local u gather from
    a padded 256B-row u table), the u-halves are AllToAll'd to the qd
    owner, which gathers v locally, adds, and takes log_softmax.
"""

import math
from contextlib import ExitStack

import ml_dtypes
import numpy as np

import concourse.bass as bass
import concourse.tile as tile
from concourse import bacc, mybir
from concourse.masks import make_identity

BF16 = mybir.dt.bfloat16
F32 = mybir.dt.float32
F32R = mybir.dt.float32r
I16 = mybir.dt.int16
P = 128
WCOLS = 16      # columns per window
GW = 32         # windows per PSUM group (512 cols)


def _wrap_idx(stream):
    """[n] int -> [128, n//16] int16 in dma_gather wrapped layout."""
    n = len(stream)
    a = np.asarray(stream, np.int64).reshape(n // 16, 16).T
    return np.tile(a, (8, 1)).astype(np.int16)


def plan(inputs, n_cores=8, verbose=False):
    x = np.asarray(inputs["x"], np.float32)
    edge_index = np.asarray(inputs["edge_index"], np.int64)
    query_edges = np.asarray(inputs["query_edges"], np.int64)
    edge_weight = np.asarray(inputs["edge_weight"], np.float32)
    W1 = np.asarray(inputs["W1"], np.float32)
    b1 = np.asarray(inputs["b1"], np.float32)
    W2 = np.asarray(inputs["W2"], np.float32)
    b2 = np.asarray(inputs["b2"], np.float32)
    Wl = np.asarray(inputs["Wl"], np.float32)
    bl = np.asarray(inputs["bl"], np.float32)

    N, F = x.shape
    E = edge_index.shape[1]
    Q = query_edges.shape[0]
    assert F == 256 and N % n_cores == 0
    n_local = N // n_cores
    NH = N // 2                      # x table split point
    cg = [0, 3 * n_local * 3 // 3, 0, 0]  # placeholder
    # owner-core groups for the L2 table third-split: {0,1,2},{3,4,5},{6,7}
    g_of_core = np.array([0, 0, 0, 1, 1, 1, 2, 2][:n_cores])
    tb = [np.searchsorted(g_of_core, t) * n_local for t in range(3)]
    tb.append(N)  # third t covers nodes [tb[t], tb[t+1])

    src = edge_index[0]
    dst = edge_index[1]
    qs, qd = query_edges[:, 0], query_edges[:, 1]

    # ---- pack windows per core (FFD by degree desc) ----
    # Separate packings for L1 (caps: lo/hi src half) and L2 (caps: src
    # owner-core third) — co-packing both forced L2 fill down to ~0.6.
    CAP = 126

    def ffd(caps):
        n_local = caps.shape[0]
        deg = caps.sum(1)
        order = np.argsort(-deg, kind="stable")
        win_of = np.empty(n_local, np.int64)
        rank_of = np.empty(n_local, np.int64)
        wins_used = []
        wins_n = []
        SCAN = 24
        for d in order:
            cd = caps[d]
            placed = False
            for wi in range(len(wins_used) - 1, max(-1, len(wins_used) - 1 - SCAN), -1):
                if wins_n[wi] < WCOLS and np.all(wins_used[wi] + cd <= CAP):
                    win_of[d] = wi
                    rank_of[d] = wins_n[wi]
                    wins_used[wi] += cd
                    wins_n[wi] += 1
                    placed = True
                    break
            if not placed:
                win_of[d] = len(wins_used)
                rank_of[d] = 0
                wins_used.append(cd.copy())
                wins_n.append(1)
        return win_of, rank_of, len(wins_used)

    packs = []
    nW = 0
    nW2 = 0
    for c in range(n_cores):
        m = dst // n_local == c
        ed = dst[m] - c * n_local
        es = src[m]
        deg = np.bincount(ed, minlength=n_local)
        lo1 = np.bincount(ed[es < NH], minlength=n_local)
        t0 = np.bincount(ed[es < tb[1]], minlength=n_local)
        t1 = np.bincount(ed[(es >= tb[1]) & (es < tb[2])], minlength=n_local)
        hi1 = deg - lo1
        t2 = deg - t0 - t1
        assert max(lo1.max(), hi1.max(), t0.max(), t1.max(), t2.max()) <= CAP
        win1, rank1, w1 = ffd(np.stack([lo1, hi1], 1))
        win2, rank2, w2 = ffd(np.stack([t0, t1, t2], 1))
        packs.append((m, win1, rank1, win2, rank2))
        nW = max(nW, w1)
        nW2 = max(nW2, w2)
    nW = ((nW + GW - 1) // GW) * GW
    nW2 = ((nW2 + GW - 1) // GW) * GW
    COLS = WCOLS * nW
    COLS2 = WCOLS * nW2
    NT = COLS // P
    NT2 = COLS2 // P
    assert 3 * COLS < 2 ** 15, "L2 table third exceeds int16 range"
    n_grp = nW // GW
    n_grp2 = nW2 // GW

    # column & global row of every node (L1 layout feeds the yab tables;
    # L2 layout feeds u/v tables and queries)
    col_all = np.empty(N, np.int64)
    g_row = np.empty(N, np.int64)
    col2_all = np.empty(N, np.int64)
    for c in range(n_cores):
        m, win1, rank1, win2, rank2 = packs[c]
        col = win1 * WCOLS + rank1
        col_all[c * n_local:(c + 1) * n_local] = col
        g_row[c * n_local:(c + 1) * n_local] = \
            c * COLS + (col % P) * NT + col // P
        col2_all[c * n_local:(c + 1) * n_local] = win2 * WCOLS + rank2

    # ---- per-core edge streams ----
    i1_l, s1_l, i2_l, s2_l = [], [], [], []
    for c in range(n_cores):
        m, win1, rank1, win2, rank2 = packs[c]
        es, ew = src[m], edge_weight[m]
        ed = dst[m] - c * n_local
        half1 = (es >= NH).astype(np.int64)
        third = np.searchsorted(np.array(tb[1:3]), es, side="right")

        def build(nsub, sub, base_vals, ewin, erank, ngrp):
            # block of edge = grp*(nsub*GW) + sub*GW + (win % GW)
            grp = ewin // GW
            blk = grp * (nsub * GW) + sub * GW + (ewin % GW)
            nblk = ngrp * nsub * GW
            # slot within block: stable order by (blk), cumcount
            order_e = np.lexsort((np.arange(len(es)), blk))
            bsort = blk[order_e]
            first = np.concatenate([[True], bsort[1:] != bsort[:-1]])
            start_pos = np.maximum.accumulate(
                np.where(first, np.arange(len(es)), 0))
            slot_sorted = np.arange(len(es)) - start_pos
            slot = np.empty(len(es), np.int64)
            slot[order_e] = slot_sorted
            assert slot.max(initial=0) < P
            idx = np.zeros((nblk, P), np.int64)
            S = np.zeros((nblk, P, WCOLS), np.float32)
            idx[blk, slot] = base_vals
            S[blk, slot, erank] = ew
            return idx, S

        sub1 = half1
        base1 = np.where(es < NH, es, es - NH)
        idx1, S1 = build(2, sub1, base1, win1[ed], rank1[ed], n_grp)
        base2 = g_row[es] - np.array([0, tb[1] // n_local * COLS,
                                      tb[2] // n_local * COLS])[third]
        idx2, S2 = build(3, third, base2, win2[ed], rank2[ed], n_grp2)

        # wrapped per-call idx [ncalls, 128, GW*P/16], S [ncalls, 128, GW, 16]
        def to_calls(idx, S, nsub, ngrp):
            ncalls = ngrp * nsub
            iw = np.empty((ncalls, P, GW * P // 16), np.int16)
            sw = np.empty((ncalls, P, GW, WCOLS), ml_dtypes.bfloat16)
            for call in range(ncalls):
                blocks = idx[call * GW:(call + 1) * GW]      # [GW, P]
                stream = blocks.reshape(GW * P)              # pos j*128+p -> block j slot p
                iw[call] = _wrap_idx(stream)
                sblk = S[call * GW:(call + 1) * GW]          # [GW, P, 16]
                sw[call] = sblk.transpose(1, 0, 2).astype(ml_dtypes.bfloat16)
            return iw, sw

        iw1, sw1 = to_calls(idx1, S1, 2, n_grp)
        iw2, sw2 = to_calls(idx2, S2, 3, n_grp2)
        i1_l.append(iw1)
        s1_l.append(sw1)
        i2_l.append(iw2)
        s2_l.append(sw2)

    # ---- queries: gather u at owner(qs), AllToAll to owner(qd) ----
    q_owner_s = qs // n_local
    q_owner_d = qd // n_local
    counts = np.zeros((n_cores, n_cores), np.int64)
    np.add.at(counts, (q_owner_s, q_owner_d), 1)
    QSLOT = ((int(counts.max()) + P - 1) // P) * P
    QTOT = n_cores * QSLOT
    QJ = QTOT // P
    loc_row = (col2_all % P) * NT2 + col2_all // P  # local u/v table row of node

    qu_l, qv_l = [], []
    send_pos = np.empty(Q, np.int64)   # position in sender's stream
    for c in range(n_cores):
        mine = np.nonzero(q_owner_s == c)[0]
        dests = q_owner_d[mine]
        order = np.argsort(dests, kind="stable")
        mine = mine[order]
        dests = dests[order]
        qu = np.zeros(QTOT, np.int64)
        fill = np.zeros(n_cores, np.int64)
        pos = np.empty(len(mine), np.int64)
        for ii, (q, d) in enumerate(zip(mine, dests)):
            pos[ii] = d * QSLOT + fill[d]
            fill[d] += 1
        qu[pos] = loc_row[qs[mine]]
        send_pos[mine] = pos
        qu_l.append(_wrap_idx(qu))
    # receiver side: position in a2a_out = s*QSLOT + slot
    qv_l = []
    out_map = []  # per core: array [QTOT] of orig query index or -1
    for c in range(n_cores):
        qv = np.zeros(QTOT, np.int64)
        omap = np.full(QTOT, -1, np.int64)
        for s in range(n_cores):
            sel = np.nonzero((q_owner_s == s) & (q_owner_d == c))[0]
            # slots assigned in sender order
            slots = send_pos[sel] - c * QSLOT  # slot within bucket
            qv[s * QSLOT + slots] = loc_row[qd[sel]]
            omap[s * QSLOT + slots] = sel
        qv_l.append(_wrap_idx(qv))
        out_map.append(omap)

    # ---- weights / constants ----
    AB = np.concatenate([W2 @ Wl[:256], W2 @ Wl[256:]], axis=1)  # [256,4]
    cu = b2 @ Wl[:256]
    cv = b2 @ Wl[256:] + bl
    cuv = np.concatenate([cu, cv]).reshape(4, 1).astype(np.float32)
    b1c = b1.reshape(2, P).astype(np.float32)
    x_bf = x.astype(ml_dtypes.bfloat16)
    w1_f = np.ascontiguousarray(W1.astype(np.float32))
    ab_f = np.ascontiguousarray(AB.astype(ml_dtypes.bfloat16))

    in_maps = []
    for c in range(n_cores):
        in_maps.append({
            "x": x_bf, "i1": i1_l[c], "s1": s1_l[c],
            "i2": i2_l[c], "s2": s2_l[c],
            "qu": qu_l[c], "qv": qv_l[c],
            "w1": w1_f, "ab": ab_f, "b1": b1c, "cuv": cuv,
        })

    dims = dict(N=N, NH=NH, W=nW, COLS=COLS, NT=NT, QJ=QJ, QSLOT=QSLOT,
                n_grp=n_grp, n_cores=n_cores,
                COLS2=COLS2, NT2=NT2, n_grp2=n_grp2,
                tsplit=(tb[1] // n_local, tb[2] // n_local))
    if verbose:
        fill1 = E / (n_cores * n_grp * 2 * GW * P)
        fill2 = E / (n_cores * n_grp2 * 3 * GW * P)
        print(f"plan: W={nW} W2={nW2} COLS={COLS} COLS2={COLS2} QSLOT={QSLOT} "
              f"QJ={QJ} fill1={fill1:.3f} fill2={fill2:.3f}")
    meta = dict(out_map=out_map, Q=Q, QJ=QJ)
    return dims, in_maps, meta


def unshard(results, meta):
    Q, QJ = meta["Q"], meta["QJ"]
    out = np.empty((Q, 2), np.float32)
    for c, res in enumerate(results):
        o = res["out"].reshape(P * QJ, 2)
        omap = meta["out_map"][c]
        # out rows: position pi lives at (p=pi%128, j=pi//128) -> flat p*QJ+j
        pi = np.nonzero(omap >= 0)[0]
        out[omap[pi]] = o[(pi % P) * QJ + pi // P]
    return out


# ----------------------------------------------------------------------------
# Device graph
# ----------------------------------------------------------------------------

def build_nc(dims):
    n_cores = dims["n_cores"]
    N, NH, COLS, NT, QJ = (dims["N"], dims["NH"], dims["COLS"], dims["NT"],
                           dims["QJ"])
    n_grp = dims["n_grp"]
    n_grp2, COLS2 = dims["n_grp2"], dims["COLS2"]
    QTOT = QJ * P

    nc = bacc.Bacc("TRN2", target_bir_lowering=False, debug=False,
                   enable_asserts=False, num_devices=n_cores,
                   num_swdge_queues=4)

    IW = GW * P // 16
    t_x = nc.dram_tensor("x", [N, 256], BF16, kind="ExternalInput")
    t_i1 = nc.dram_tensor("i1", [n_grp * 2, P, IW], I16, kind="ExternalInput")
    t_s1 = nc.dram_tensor("s1", [n_grp * 2, P, GW, WCOLS], BF16,
                          kind="ExternalInput")
    t_i2 = nc.dram_tensor("i2", [n_grp2 * 3, P, IW], I16, kind="ExternalInput")
    t_s2 = nc.dram_tensor("s2", [n_grp2 * 3, P, GW, WCOLS], BF16,
                          kind="ExternalInput")
    t_qu = nc.dram_tensor("qu", [P, QTOT // 16], I16, kind="ExternalInput")
    t_qv = nc.dram_tensor("qv", [P, QTOT // 16], I16, kind="ExternalInput")
    t_w1 = nc.dram_tensor("w1", [256, 256], F32, kind="ExternalInput")
    t_ab = nc.dram_tensor("ab", [256, 4], BF16, kind="ExternalInput")
    t_b1 = nc.dram_tensor("b1", [2, P], F32, kind="ExternalInput")
    t_cuv = nc.dram_tensor("cuv", [4, 1], F32, kind="ExternalInput")
    t_out = nc.dram_tensor("out", [P, QJ, 2], F32, kind="ExternalOutput")

    t_yab = nc.dram_tensor("yab_l", [P, NT * 4], BF16)
    t_uvc = nc.dram_tensor("uvc", [n_cores * P, NT * 4], BF16,
                           addr_space="Shared")
    c0, c1 = dims["tsplit"]
    t_uvp0 = nc.dram_tensor("uvp0", [c0 * COLS, P], BF16)
    t_uvp1 = nc.dram_tensor("uvp1", [(c1 - c0) * COLS, P], BF16)
    t_uvp2 = nc.dram_tensor("uvp2", [(n_cores - c1) * COLS, P], BF16)
    t_upad = nc.dram_tensor("upad", [COLS2, 64], F32)
    t_vpad = nc.dram_tensor("vpad", [COLS2, 64], F32)
    t_a2i = nc.dram_tensor("a2i", [QTOT, 2], F32)
    t_a2o = nc.dram_tensor("a2o", [QTOT, 2], F32)

    tensors = locals()
    with tile.TileContext(nc) as tc:
        with ExitStack() as ctx:
            _emit(ctx, tc, dims, tensors)
    nc.compile()
    return nc


def _emit(ctx, tc, dims, T):
    nc = tc.nc
    n_cores = dims["n_cores"]
    N, NH, COLS, NT, QJ = (dims["N"], dims["NH"], dims["COLS"], dims["NT"],
                           dims["QJ"])
    n_grp = dims["n_grp"]
    n_grp2, COLS2, NT2 = dims["n_grp2"], dims["COLS2"], dims["NT2"]
    c0, c1 = dims["tsplit"]
    QTOT = QJ * P
    IW = GW * P // 16
    NI = GW * P
    Relu = mybir.ActivationFunctionType.Relu
    Copy = mybir.ActivationFunctionType.Copy
    Exp = mybir.ActivationFunctionType.Exp
    Ln = mybir.ActivationFunctionType.Ln

    const = ctx.enter_context(tc.tile_pool(name="const", bufs=1))

    w1A = const.tile([P, 256], F32)
    nc.sync.dma_start(w1A[:], T["t_w1"].ap()[0:P, :])
    w1B = const.tile([P, 256], F32)
    nc.sync.dma_start(w1B[:], T["t_w1"].ap()[P:256, :])
    w1Ar = const.tile([P, 256], F32R)
    nc.vector.tensor_copy(w1Ar[:], w1A[:])
    w1Br = const.tile([P, 256], F32R)
    nc.vector.tensor_copy(w1Br[:], w1B[:])
    abA = const.tile([P, 4], BF16)
    nc.sync.dma_start(abA[:], T["t_ab"].ap()[0:P, :])
    abB = const.tile([P, 4], BF16)
    nc.sync.dma_start(abB[:], T["t_ab"].ap()[P:256, :])
    b1A = const.tile([P, 1], F32)
    nc.sync.dma_start(b1A[:], T["t_b1"].ap()[0, :, None])
    b1B = const.tile([P, 1], F32)
    nc.sync.dma_start(b1B[:], T["t_b1"].ap()[1, :, None])
    cuv = const.tile([4, 1], F32)
    nc.sync.dma_start(cuv[:], T["t_cuv"].ap()[:, :])
    qu = const.tile([P, QTOT // 16], I16)
    nc.sync.dma_start(qu[:], T["t_qu"].ap()[:, :])
    qv = const.tile([P, QTOT // 16], I16)
    nc.sync.dma_start(qv[:], T["t_qv"].ap()[:, :])
    id4 = const.tile([4, 4], F32)
    make_identity(nc, id4[:])

    # long-lived tail tiles (before h1p: pool closes stay LIFO)
    tail = ctx.enter_context(tc.tile_pool(name="tail", bufs=1))
    ystage = tail.tile([P, NT, 4], BF16)
    uvT = tail.tile([4, COLS2], F32)
    uvn = tail.tile([P, NT2, 4], F32)

    h1pool_cm = tc.tile_pool(name="h1p", bufs=1)
    h1pool = h1pool_cm.__enter__()
    h1A = h1pool.tile([P, COLS], BF16)
    h1B = h1pool.tile([P, COLS], BF16)

    x_views = [T["t_x"].ap()[0:NH, :], T["t_x"].ap()[NH:N, :]]

    # ---------------- layer 1 (+ yab = h1 @ [A|B] folded per group) -------
    with tc.tile_pool(name="msgs", bufs=3) as msgs_pool, \
         tc.tile_pool(name="idxp", bufs=3) as idxp, \
         tc.tile_pool(name="sp", bufs=3) as sp, \
         tc.tile_pool(name="aggp", bufs=3) as aggp, \
         tc.tile_pool(name="ps1", bufs=2, space="PSUM") as ps1, \
         tc.tile_pool(name="ps1b", bufs=2, space="PSUM") as ps1b, \
         tc.tile_pool(name="psz", bufs=2, space="PSUM") as psz, \
         tc.tile_pool(name="psy", bufs=1, space="PSUM") as psy, \
         tc.tile_pool(name="psyt", bufs=1, space="PSUM") as psyt:
        for g in range(n_grp):
            agA = aggp.tile([P, GW * WCOLS], F32R, tag="agA")
            agB = aggp.tile([P, GW * WCOLS], F32R, tag="agB")
            for half in range(2):
                call = g * 2 + half
                idxt = idxp.tile([P, IW], I16, tag="ix")
                nc.sync.dma_start(idxt[:], T["t_i1"].ap()[call, :, :])
                st = sp.tile([P, GW, WCOLS], BF16, tag="s")
                nc.sync.dma_start(st[:], T["t_s1"].ap()[call, :, :, :])
                mts = []
                for s in range(NI // 1024):
                    mt = msgs_pool.tile([P, 8, 256], BF16, tag=f"m1_{s}")
                    nc.gpsimd.dma_gather(
                        mt[:], x_views[half],
                        idxt[:, 64 * s:64 * (s + 1)], 1024, 1024, 256,
                        single_packet=False, queue_num=s)
                    mts.append(mt)
                pA = ps1.tile([P, GW * WCOLS], F32, tag="pA")
                pB = ps1b.tile([P, GW * WCOLS], F32, tag="pB")
                for j in range(GW):
                    cs = slice(WCOLS * j, WCOLS * (j + 1))
                    mt = mts[j // 8]
                    jj = j % 8
                    nc.tensor.matmul(pA[:, cs], lhsT=mt[:, jj, 0:P],
                                     rhs=st[:, j, :],
                                     start=(j == 0), stop=(j == GW - 1))
                    nc.tensor.matmul(pB[:, cs], lhsT=mt[:, jj, P:256],
                                     rhs=st[:, j, :],
                                     start=(j == 0), stop=(j == GW - 1))
                if half == 0:
                    nc.scalar.activation(agA[:], pA[:], Copy)
                    nc.vector.tensor_copy(agB[:], pB[:])
                else:
                    nc.vector.tensor_tensor(agA[:], agA[:], pA[:],
                                            op=mybir.AluOpType.add)
                    nc.vector.tensor_tensor(agB[:], agB[:], pB[:],
                                            op=mybir.AluOpType.add)
            cols = slice(g * GW * WCOLS, (g + 1) * GW * WCOLS)
            for m in range(2):
                pz = psz.tile([P, GW * WCOLS], F32, tag="pz")
                nc.tensor.matmul(pz[:], lhsT=w1Ar[:, m * P:(m + 1) * P],
                                 rhs=agA[:], start=True, stop=False)
                nc.tensor.matmul(pz[:], lhsT=w1Br[:, m * P:(m + 1) * P],
                                 rhs=agB[:], start=False, stop=True)
                h1m = h1A if m == 0 else h1B
                b1m = b1A if m == 0 else b1B
                nc.scalar.activation(h1m[:, cols], pz[:], Relu,
                                     bias=b1m[:, 0:1])
            # yab chunk for this group: yT[4, 512] = abA.T@h1A + abB.T@h1B
            py = psy.tile([4, GW * WCOLS], F32, tag="py")
            nc.tensor.matmul(py[:], lhsT=abA[:], rhs=h1A[:, cols],
                             start=True, stop=False)
            nc.tensor.matmul(py[:], lhsT=abB[:], rhs=h1B[:, cols],
                             start=False, stop=True)
            ysb = aggp.tile([4, GW * WCOLS], F32, tag="ysb")
            nc.scalar.activation(ysb[:], py[:], Copy)
            for tt in range(GW * WCOLS // P):
                pyt = psyt.tile([P, 4], F32, tag="pyt")
                nc.tensor.transpose(pyt[:], ysb[:, tt * P:(tt + 1) * P], id4[:])
                nc.vector.tensor_copy(
                    ystage[:, g * (GW * WCOLS // P) + tt, :], pyt[:])
    nc.sync.dma_start(T["t_yab"].ap().rearrange("p (t c) -> p t c", c=4),
                      ystage[:])
    h1pool_cm.__exit__(None, None, None)

    # ---------------- AllGather yab + pad-spray ----------------
    nc.gpsimd.collective_compute(
        "AllGather", mybir.AluOpType.bypass,
        replica_groups=[list(range(n_cores))],
        ins=[T["t_yab"].ap().opt()],
        outs=[T["t_uvc"].ap().opt()],
    )
    uvc_rows = T["t_uvc"].ap().rearrange("a (b c) -> (a b) c", c=4)
    third_starts = [0, c0 * COLS, c1 * COLS, n_cores * COLS]
    uvp_t = [T["t_uvp0"], T["t_uvp1"], T["t_uvp2"]]
    # spray each third in two chunks across the two HWDGE engines (6 streams)
    spray_eng = [nc.sync, nc.scalar]
    for t in range(3):
        nrow = third_starts[t + 1] - third_starts[t]
        hcut = (nrow // 2) // P * P
        for hh, (r0, r1) in enumerate(((0, hcut), (hcut, nrow))):
            spray_eng[hh].dma_start(
                uvp_t[t].ap()[r0:r1, 0:4],
                uvc_rows[third_starts[t] + r0:third_starts[t] + r1, :])
    tuv_views = [uvp_t[t].ap()[:, :] for t in range(3)]

    # ---------------- layer 2 ----------------
    with tc.tile_pool(name="m2", bufs=4) as m2pool, \
         tc.tile_pool(name="idxp2", bufs=4) as idxp2, \
         tc.tile_pool(name="sp2", bufs=4) as sp2, \
         tc.tile_pool(name="ps2", bufs=2, space="PSUM") as ps2:
        for g in range(n_grp2):
            puv = ps2.tile([4, GW * WCOLS], F32, tag="puv")
            for third in range(3):
                call = g * 3 + third
                idxt = idxp2.tile([P, IW], I16, tag="ix2")
                nc.sync.dma_start(idxt[:], T["t_i2"].ap()[call, :, :])
                st = sp2.tile([P, GW, WCOLS], BF16, tag="s2")
                nc.sync.dma_start(st[:], T["t_s2"].ap()[call, :, :, :])
                mt2s = []
                for s in range(NI // 1024):
                    mt2 = m2pool.tile([P, 8, P], BF16, tag=f"m2_{s}")
                    nc.gpsimd.dma_gather(
                        mt2[:], tuv_views[third],
                        idxt[:, 64 * s:64 * (s + 1)], 1024, 1024, P,
                        single_packet=False, queue_num=s)
                    mt2s.append(mt2)
                for j in range(GW):
                    cs = slice(WCOLS * j, WCOLS * (j + 1))
                    nc.tensor.matmul(puv[:, cs], lhsT=mt2s[j // 8][:, j % 8, 0:4],
                                     rhs=st[:, j, :],
                                     start=(third == 0 and j == 0),
                                     stop=(third == 2 and j == GW - 1))
            nc.vector.tensor_tensor(
                uvT[:, g * GW * WCOLS:(g + 1) * GW * WCOLS], puv[:],
                cuv[:, 0:1].to_broadcast([4, GW * WCOLS]),
                op=mybir.AluOpType.add)

    # ---------------- transpose uvT -> node-major, build u/v tables -------
    with tc.tile_pool(name="pst", bufs=2, space="PSUM") as pst:
        for t in range(NT2):
            ptp = pst.tile([P, 4], F32, tag="ptp")
            nc.tensor.transpose(ptp[:], uvT[:, t * P:(t + 1) * P], id4[:])
            nc.vector.tensor_copy(uvn[:, t, :], ptp[:])
    upad_rows = T["t_upad"].ap()[:, 0:2].rearrange("(p t) c -> p t c", p=P)
    vpad_rows = T["t_vpad"].ap()[:, 0:2].rearrange("(p t) c -> p t c", p=P)
    nc.sync.dma_start(upad_rows, uvn[:, :, 0:2])
    nc.sync.dma_start(vpad_rows, uvn[:, :, 2:4])

    # ---------------- query head ----------------
    qp = ctx.enter_context(tc.tile_pool(name="qp", bufs=1))
    ug = qp.tile([P, QJ, 64], F32)
    for s in range(QTOT // 1024):
        nc.gpsimd.dma_gather(
            ug[:, 8 * s:8 * (s + 1), :], T["t_upad"].ap()[:, :],
            qu[:, 64 * s:64 * (s + 1)], 1024, 1024, 64,
            single_packet=False, queue_num=s % 4)
    us = qp.tile([P, QJ, 2], F32)
    nc.vector.tensor_copy(us[:], ug[:, :, 0:2])
    a2i_v = T["t_a2i"].ap().rearrange("(j p) c -> p j c", p=P)
    nc.sync.dma_start(a2i_v, us[:])
    vg = qp.tile([P, QJ, 64], F32)
    for s in range(QTOT // 1024):
        nc.gpsimd.dma_gather(
            vg[:, 8 * s:8 * (s + 1), :], T["t_vpad"].ap()[:, :],
            qv[:, 64 * s:64 * (s + 1)], 1024, 1024, 64,
            single_packet=False, queue_num=s % 4)
    nc.gpsimd.collective_compute(
        "AllToAll", mybir.AluOpType.bypass,
        replica_groups=[list(range(n_cores))],
        ins=[T["t_a2i"].ap().opt()],
        outs=[T["t_a2o"].ap().opt()],
    )
    ut2 = qp.tile([P, QJ, 2], F32)
    a2o_v = T["t_a2o"].ap().rearrange("(j p) c -> p j c", p=P)
    nc.sync.dma_start(ut2[:], a2o_v)

    lg = qp.tile([P, QJ, 2], F32)
    nc.vector.tensor_tensor(lg[:], ut2[:], vg[:, :, 0:2],
                            op=mybir.AluOpType.add)
    mx = qp.tile([P, QJ, 1], F32)
    nc.vector.reduce_max(mx[:], lg[:], axis=mybir.AxisListType.X)
    tt = qp.tile([P, QJ, 2], F32)
    nc.vector.tensor_tensor(tt[:], lg[:], mx[:].to_broadcast([P, QJ, 2]),
                            op=mybir.AluOpType.subtract)
    ex = qp.tile([P, QJ, 2], F32)
    nc.scalar.activation(ex[:], tt[:], Exp)
    sm = qp.tile([P, QJ, 1], F32)
    nc.vector.reduce_sum(sm[:], ex[:], axis=mybir.AxisListType.X)
    ls = qp.tile([P, QJ, 1], F32)
    nc.scalar.activation(ls[:], sm[:], Ln)
    oo = qp.tile([P, QJ, 2], F32)
    nc.vector.tensor_tensor(oo[:], tt[:], ls[:].to_broadcast([P, QJ, 2]),
                            op=mybir.AluOpType.subtract)
    nc.sync.dma_start(T["t_out"].ap()[:, :, :], oo[:])


# ----------------------------------------------------------------------------
# numpy reference (mirrors reference.py math in f32)
# ----------------------------------------------------------------------------

def numpy_reference(inputs):
    x = np.asarray(inputs["x"], np.float32)
    ei = np.asarray(inputs["edge_index"], np.int64)
    qe = np.asarray(inputs["query_edges"], np.int64)
    w = np.asarray(inputs["edge_weight"], np.float32)
    W1, b1 = np.asarray(inputs["W1"], np.float32), np.asarray(inputs["b1"], np.float32)
    W2, b2 = np.asarray(inputs["W2"], np.float32), np.asarray(inputs["b2"], np.float32)
    Wl, bl = np.asarray(inputs["Wl"], np.float32), np.asarray(inputs["bl"], np.float32)
    N = x.shape[0]
    src, dst = ei[0], ei[1]

    def conv(h, W, b):
        z = h @ W
        msg = z[src] * w[:, None]
        agg = np.zeros((N, z.shape[1]), np.float32)
        np.add.at(agg, dst, msg)
        return agg + b

    h1 = np.maximum(conv(x, W1, b1), 0.0)
    h2 = conv(h1, W2, b2)
    q = np.concatenate([h2[qe[:, 0]], h2[qe[:, 1]]], axis=1)
    logits = q @ Wl + bl
    m = logits.max(axis=1, keepdims=True)
    e = np.exp(logits - m)
    return logits - m - np.log(e.sum(axis=1, keepdims=True))

# ----------------------------------------------------------------------------
# Entry point: full inputs in, full output out
# ----------------------------------------------------------------------------

LAST_RESULTS = None


def kernel(**inputs):
    """Takes the FULL (unsharded) inputs of nn_DiGCN_link_prediction and
    returns the full [N_QUERY, 2] float32 log-softmax output.

    Shards nodes/edges/queries across 8 NeuronCores internally, runs one
    SPMD Bass kernel (per-core data, identical graph), and reassembles.
    """
    global LAST_RESULTS
    import os
    from concourse.bass_utils import run_bass_kernel_spmd

    n_cores = 8
    dims, in_maps, meta = plan(inputs, n_cores=n_cores)
    nc = build_nc(dims)
    res = run_bass_kernel_spmd(
        nc, in_maps, core_ids=list(range(n_cores)),
        trace=bool(int(os.environ.get("GNN_TRACE", "0"))),
        stitch_traces=False,
    )
    LAST_RESULTS = res
    return unshard(res.results, meta)



# revision 17
# speedup vs baseline: 1.7507x; 1.4651x over previous
"""DiGCN link prediction on 8 TRN2 NeuronCores.

Math (reference):
    h1 = relu(segsum_dst(w_e * (x@W1)[src]) + b1)
    h2 = segsum_dst(w_e * (h1@W2)[src]) + b2
    logits = concat(h2[qs], h2[qd]) @ Wl + bl ; out = log_softmax(logits)

Device strategy (per core, SPMD-identical graph, per-core data):
  - dst-sharded edges. Host packs each core's dsts into fixed 16-column
    windows (whole dsts, FFD), 32 windows per 512-col PSUM group.
  - Layer 1 uses linearity: segsum(w, x@W1) = segsum(w, x)@W1. Per window
    two 128-slot gather blocks (src<25000 and src>=25000 halves, int16
    dma_gather from the two x table halves), per-block matmul
    msgs^T[128e,128f] @ S[128e,16] into feature-major PSUM; the hi pass
    adds on top in SBUF. Then project W1^T (f32r) + relu + b1 -> h1T.
  - Layer 2 + head use linearity again: with A=W2@Wl[:256], B=W2@Wl[256:],
    u[d]=sum w_e*(h1@A)[src]+b2@Wlt, v[d]=sum w_e*(h1@B)[src]+b2@Wlb+bl,
    logits[q] = u[qs]+v[qd]. yab=h1@[A|B] ([*,4] bf16) is AllGathered
    (1 MB), padded locally into a 256B-row table, aggregated with the same
    window structure (3 sub-blocks per window, split by src owner-core
    group for int16).
  - Query head: queries are processed where qs lives (local u gather from
    a padded 256B-row u table), the u-halves are AllToAll'd to the qd
    owner, which gathers v locally, adds, and takes log_softmax.
"""

import math
from contextlib import ExitStack

import ml_dtypes
import numpy as np

import concourse.bass as bass
import concourse.tile as tile
from concourse import bacc, mybir
from concourse.masks import make_identity

BF16 = mybir.dt.bfloat16
F32 = mybir.dt.float32
F32R = mybir.dt.float32r
I16 = mybir.dt.int16
P = 128
WCOLS = 16      # columns per window
GW = 32         # windows per PSUM group (512 cols)


def _wrap_idx(stream):
    """[n] int -> [128, n//16] int16 in dma_gather wrapped layout."""
    n = len(stream)
    a = np.asarray(stream, np.int64).reshape(n // 16, 16).T
    return np.tile(a, (8, 1)).astype(np.int16)


def plan(inputs, n_cores=8, verbose=False):
    x = np.asarray(inputs["x"], np.float32)
    edge_index = np.asarray(inputs["edge_index"], np.int64)
    query_edges = np.asarray(inputs["query_edges"], np.int64)
    edge_weight = np.asarray(inputs["edge_weight"], np.float32)
    W1 = np.asarray(inputs["W1"], np.float32)
    b1 = np.asarray(inputs["b1"], np.float32)
    W2 = np.asarray(inputs["W2"], np.float32)
    b2 = np.asarray(inputs["b2"], np.float32)
    Wl = np.asarray(inputs["Wl"], np.float32)
    bl = np.asarray(inputs["bl"], np.float32)

    N, F = x.shape
    E = edge_index.shape[1]
    Q = query_edges.shape[0]
    assert F == 256 and N % n_cores == 0
    n_local = N // n_cores
    NH = N // 2                      # x table split point (= owner-half split)

    src = edge_index[0]
    dst = edge_index[1]
    qs, qd = query_edges[:, 0], query_edges[:, 1]

    # ---- pack windows per core ----
    # One packing serves both layers: the L1 gather sub-split (src node-id
    # half) equals the L2 table sub-split (owner-core half), because cores
    # 0-3 own nodes [0, NH). Balanced worst-fit (order by max cap dim)
    # packs every core into exactly 512 windows -> COLS=8192 and the L2
    # half-table has 4*COLS = 32768 rows = exact int16 range.
    CAP = 128
    WFIX = 512

    def pack_windows(caps):
        n_loc, k = caps.shape
        order = np.argsort(-caps.max(1), kind="stable")
        floor = max((n_loc + WCOLS - 1) // WCOLS,
                    int(np.ceil(caps.sum(0).max() / CAP)))
        for Wmax in range(max(WFIX, floor), max(WFIX, floor) + 97, 8):
            used = np.zeros((Wmax, k), np.int64)
            cnt = np.zeros(Wmax, np.int64)
            win_of = np.empty(n_loc, np.int64)
            rank_of = np.empty(n_loc, np.int64)
            ok = True
            for d in order:
                cd = caps[d]
                feas = (cnt < WCOLS) & np.all(used + cd <= CAP, axis=1)
                if not feas.any():
                    ok = False
                    break
                score = (used + cd).max(axis=1).astype(np.float64) + 1e-3 * cnt
                wi = int(np.argmin(np.where(feas, score, 1e18)))
                win_of[d] = wi
                rank_of[d] = cnt[wi]
                used[wi] += cd
                cnt[wi] += 1
            if ok:
                return win_of, rank_of, Wmax
        raise RuntimeError("pack_windows failed")

    packs = []
    nW = 0
    for c in range(n_cores):
        m = dst // n_local == c
        ed = dst[m] - c * n_local
        es = src[m]
        deg = np.bincount(ed, minlength=n_local)
        lo1 = np.bincount(ed[es < NH], minlength=n_local)
        hi1 = deg - lo1
        assert max(lo1.max(), hi1.max()) <= CAP
        win1, rank1, w1 = pack_windows(np.stack([lo1, hi1], 1))
        packs.append((m, win1, rank1))
        nW = max(nW, w1)
    nW = ((nW + GW - 1) // GW) * GW
    COLS = WCOLS * nW
    NT = COLS // P
    assert n_cores // 2 * COLS <= 2 ** 15, "L2 half-table exceeds int16 range"
    n_grp = nW // GW

    # column & global row of every node (shared by both layers)
    col_all = np.empty(N, np.int64)
    g_row = np.empty(N, np.int64)
    for c in range(n_cores):
        m, win1, rank1 = packs[c]
        col = win1 * WCOLS + rank1
        col_all[c * n_local:(c + 1) * n_local] = col
        g_row[c * n_local:(c + 1) * n_local] = \
            c * COLS + (col % P) * NT + col // P

    # ---- per-core edge streams ----
    i1_l, s1_l, i2_l = [], [], []
    for c in range(n_cores):
        m, win1, rank1 = packs[c]
        es, ew = src[m], edge_weight[m]
        ed = dst[m] - c * n_local
        half1 = (es >= NH).astype(np.int64)

        def build(nsub, sub, base_vals, ewin, erank, ngrp):
            # block of edge = grp*(nsub*GW) + sub*GW + (win % GW)
            grp = ewin // GW
            blk = grp * (nsub * GW) + sub * GW + (ewin % GW)
            nblk = ngrp * nsub * GW
            # slot within block: stable order by (blk), cumcount
            order_e = np.lexsort((np.arange(len(es)), blk))
            bsort = blk[order_e]
            first = np.concatenate([[True], bsort[1:] != bsort[:-1]])
            start_pos = np.maximum.accumulate(
                np.where(first, np.arange(len(es)), 0))
            slot_sorted = np.arange(len(es)) - start_pos
            slot = np.empty(len(es), np.int64)
            slot[order_e] = slot_sorted
            assert slot.max(initial=0) < P
            idx = np.zeros((nblk, P), np.int64)
            S = np.zeros((nblk, P, WCOLS), np.float32)
            idx[blk, slot] = base_vals
            S[blk, slot, erank] = ew
            return idx, S

        sub1 = half1
        base1 = np.where(es < NH, es, es - NH)
        idx1, S1 = build(2, sub1, base1, win1[ed], rank1[ed], n_grp)
        base2 = g_row[es] - half1 * (n_cores // 2 * COLS)
        idx2, S2 = build(2, sub1, base2, win1[ed], rank1[ed], n_grp)

        # wrapped per-call idx [ncalls, 128, GW*P/16], S [ncalls, 128, GW, 16]
        def to_calls(idx, S, nsub, ngrp):
            ncalls = ngrp * nsub
            iw = np.empty((ncalls, P, GW * P // 16), np.int16)
            sw = np.empty((ncalls, P, GW, WCOLS), ml_dtypes.bfloat16)
            for call in range(ncalls):
                blocks = idx[call * GW:(call + 1) * GW]      # [GW, P]
                stream = blocks.reshape(GW * P)              # pos j*128+p -> block j slot p
                iw[call] = _wrap_idx(stream)
                sblk = S[call * GW:(call + 1) * GW]          # [GW, P, 16]
                sw[call] = sblk.transpose(1, 0, 2).astype(ml_dtypes.bfloat16)
            return iw, sw

        iw1, sw1 = to_calls(idx1, S1, 2, n_grp)
        iw2, _ = to_calls(idx2, S2, 2, n_grp)
        i1_l.append(iw1)
        s1_l.append(sw1)
        i2_l.append(iw2)

    # ---- queries: gather u at owner(qs), AllToAll to owner(qd) ----
    q_owner_s = qs // n_local
    q_owner_d = qd // n_local
    counts = np.zeros((n_cores, n_cores), np.int64)
    np.add.at(counts, (q_owner_s, q_owner_d), 1)
    QSLOT = ((int(counts.max()) + P - 1) // P) * P
    QTOT = n_cores * QSLOT
    QJ = QTOT // P
    loc_row = (col_all % P) * NT + col_all // P  # local u/v table row of node

    qu_l, qv_l = [], []
    send_pos = np.empty(Q, np.int64)   # position in sender's stream
    for c in range(n_cores):
        mine = np.nonzero(q_owner_s == c)[0]
        dests = q_owner_d[mine]
        order = np.argsort(dests, kind="stable")
        mine = mine[order]
        dests = dests[order]
        qu = np.zeros(QTOT, np.int64)
        fill = np.zeros(n_cores, np.int64)
        pos = np.empty(len(mine), np.int64)
        for ii, (q, d) in enumerate(zip(mine, dests)):
            pos[ii] = d * QSLOT + fill[d]
            fill[d] += 1
        qu[pos] = loc_row[qs[mine]]
        send_pos[mine] = pos
        qu_l.append(_wrap_idx(qu))
    # receiver side: position in a2a_out = s*QSLOT + slot
    qv_l = []
    out_map = []  # per core: array [QTOT] of orig query index or -1
    for c in range(n_cores):
        qv = np.zeros(QTOT, np.int64)
        omap = np.full(QTOT, -1, np.int64)
        for s in range(n_cores):
            sel = np.nonzero((q_owner_s == s) & (q_owner_d == c))[0]
            # slots assigned in sender order
            slots = send_pos[sel] - c * QSLOT  # slot within bucket
            qv[s * QSLOT + slots] = loc_row[qd[sel]]
            omap[s * QSLOT + slots] = sel
        qv_l.append(_wrap_idx(qv))
        out_map.append(omap)

    # ---- weights / constants ----
    AB = np.concatenate([W2 @ Wl[:256], W2 @ Wl[256:]], axis=1)  # [256,4]
    cu = b2 @ Wl[:256]
    cv = b2 @ Wl[256:] + bl
    cuv = np.concatenate([cu, cv]).reshape(4, 1).astype(np.float32)
    b1c = b1.reshape(2, P).astype(np.float32)
    x_bf = x.astype(ml_dtypes.bfloat16)
    w1_f = np.ascontiguousarray(W1.astype(np.float32))
    ab_f = np.ascontiguousarray(AB.astype(ml_dtypes.bfloat16))

    in_maps = []
    for c in range(n_cores):
        in_maps.append({
            "x": x_bf, "i1": i1_l[c], "s1": s1_l[c],
            "i2": i2_l[c],
            "qu": qu_l[c], "qv": qv_l[c],
            "w1": w1_f, "ab": ab_f, "b1": b1c, "cuv": cuv,
        })

    dims = dict(N=N, NH=NH, W=nW, COLS=COLS, NT=NT, QJ=QJ, QSLOT=QSLOT,
                n_grp=n_grp, n_cores=n_cores)
    if verbose:
        fill1 = E / (n_cores * n_grp * 2 * GW * P)
        print(f"plan: W={nW} COLS={COLS} QSLOT={QSLOT} "
              f"QJ={QJ} fill={fill1:.3f}")
    meta = dict(out_map=out_map, Q=Q, QJ=QJ)
    return dims, in_maps, meta


def unshard(results, meta):
    Q, QJ = meta["Q"], meta["QJ"]
    out = np.empty((Q, 2), np.float32)
    for c, res in enumerate(results):
        o = res["out"].reshape(P * QJ, 2)
        omap = meta["out_map"][c]
        # out rows: position pi lives at (p=pi%128, j=pi//128) -> flat p*QJ+j
        pi = np.nonzero(omap >= 0)[0]
        out[omap[pi]] = o[(pi % P) * QJ + pi // P]
    return out


# ----------------------------------------------------------------------------
# Device graph
# ----------------------------------------------------------------------------

def build_nc(dims):
    n_cores = dims["n_cores"]
    N, NH, COLS, NT, QJ = (dims["N"], dims["NH"], dims["COLS"], dims["NT"],
                           dims["QJ"])
    n_grp = dims["n_grp"]
    QTOT = QJ * P

    nc = bacc.Bacc("TRN2", target_bir_lowering=False, debug=False,
                   enable_asserts=False, num_devices=n_cores,
                   num_swdge_queues=4)

    IW = GW * P // 16
    t_x = nc.dram_tensor("x", [N, 256], BF16, kind="ExternalInput")
    t_i1 = nc.dram_tensor("i1", [n_grp * 2, P, IW], I16, kind="ExternalInput")
    t_s1 = nc.dram_tensor("s1", [n_grp * 2, P, GW, WCOLS], BF16,
                          kind="ExternalInput")
    t_i2 = nc.dram_tensor("i2", [n_grp * 2, P, IW], I16, kind="ExternalInput")
    t_qu = nc.dram_tensor("qu", [P, QTOT // 16], I16, kind="ExternalInput")
    t_qv = nc.dram_tensor("qv", [P, QTOT // 16], I16, kind="ExternalInput")
    t_w1 = nc.dram_tensor("w1", [256, 256], F32, kind="ExternalInput")
    t_ab = nc.dram_tensor("ab", [256, 4], BF16, kind="ExternalInput")
    t_b1 = nc.dram_tensor("b1", [2, P], F32, kind="ExternalInput")
    t_cuv = nc.dram_tensor("cuv", [4, 1], F32, kind="ExternalInput")
    t_out = nc.dram_tensor("out", [P, QJ, 2], F32, kind="ExternalOutput")

    t_yab = nc.dram_tensor("yab_l", [P, NT * 4], BF16)
    t_uvc = nc.dram_tensor("uvc", [n_cores * P, NT * 4], BF16,
                           addr_space="Shared")
    nch = n_cores // 2
    t_uvp0 = nc.dram_tensor("uvp0", [nch * COLS, P], BF16)
    t_uvp1 = nc.dram_tensor("uvp1", [nch * COLS, P], BF16)
    t_upad = nc.dram_tensor("upad", [COLS, 64], F32)
    t_vpad = nc.dram_tensor("vpad", [COLS, 64], F32)
    t_a2i = nc.dram_tensor("a2i", [QTOT, 2], F32)
    t_a2o = nc.dram_tensor("a2o", [QTOT, 2], F32)

    tensors = locals()
    with tile.TileContext(nc) as tc:
        with ExitStack() as ctx:
            _emit(ctx, tc, dims, tensors)
    nc.compile()
    return nc


def _emit(ctx, tc, dims, T):
    nc = tc.nc
    n_cores = dims["n_cores"]
    N, NH, COLS, NT, QJ = (dims["N"], dims["NH"], dims["COLS"], dims["NT"],
                           dims["QJ"])
    n_grp = dims["n_grp"]
    QTOT = QJ * P
    IW = GW * P // 16
    NI = GW * P
    Relu = mybir.ActivationFunctionType.Relu
    Copy = mybir.ActivationFunctionType.Copy
    Exp = mybir.ActivationFunctionType.Exp
    Ln = mybir.ActivationFunctionType.Ln

    const = ctx.enter_context(tc.tile_pool(name="const", bufs=1))

    w1A = const.tile([P, 256], F32)
    nc.sync.dma_start(w1A[:], T["t_w1"].ap()[0:P, :])
    w1B = const.tile([P, 256], F32)
    nc.sync.dma_start(w1B[:], T["t_w1"].ap()[P:256, :])
    w1Ar = const.tile([P, 256], F32R)
    nc.vector.tensor_copy(w1Ar[:], w1A[:])
    w1Br = const.tile([P, 256], F32R)
    nc.vector.tensor_copy(w1Br[:], w1B[:])
    abA = const.tile([P, 4], BF16)
    nc.sync.dma_start(abA[:], T["t_ab"].ap()[0:P, :])
    abB = const.tile([P, 4], BF16)
    nc.sync.dma_start(abB[:], T["t_ab"].ap()[P:256, :])
    b1A = const.tile([P, 1], F32)
    nc.sync.dma_start(b1A[:], T["t_b1"].ap()[0, :, None])
    b1B = const.tile([P, 1], F32)
    nc.sync.dma_start(b1B[:], T["t_b1"].ap()[1, :, None])
    cuv = const.tile([4, 1], F32)
    nc.sync.dma_start(cuv[:], T["t_cuv"].ap()[:, :])
    qu = const.tile([P, QTOT // 16], I16)
    nc.sync.dma_start(qu[:], T["t_qu"].ap()[:, :])
    qv = const.tile([P, QTOT // 16], I16)
    nc.sync.dma_start(qv[:], T["t_qv"].ap()[:, :])
    id4 = const.tile([4, 4], F32)
    make_identity(nc, id4[:])

    # long-lived tail tiles (before h1p: pool closes stay LIFO)
    tail = ctx.enter_context(tc.tile_pool(name="tail", bufs=1))
    ystage = tail.tile([P, NT, 4], BF16)
    uvT = tail.tile([4, COLS], F32)
    uvn = tail.tile([P, NT, 4], F32)

    h1pool_cm = tc.tile_pool(name="h1p", bufs=1)
    h1pool = h1pool_cm.__enter__()
    h1A = h1pool.tile([P, COLS], BF16)
    h1B = h1pool.tile([P, COLS], BF16)

    x_views = [T["t_x"].ap()[0:NH, :], T["t_x"].ap()[NH:N, :]]

    # ---------------- layer 1 (+ yab = h1 @ [A|B] folded per group) -------
    with tc.tile_pool(name="msgs", bufs=3) as msgs_pool, \
         tc.tile_pool(name="idxp", bufs=3) as idxp, \
         tc.tile_pool(name="sp", bufs=3) as sp, \
         tc.tile_pool(name="aggp", bufs=3) as aggp, \
         tc.tile_pool(name="ps1", bufs=2, space="PSUM") as ps1, \
         tc.tile_pool(name="ps1b", bufs=2, space="PSUM") as ps1b, \
         tc.tile_pool(name="psz", bufs=2, space="PSUM") as psz, \
         tc.tile_pool(name="psy", bufs=1, space="PSUM") as psy, \
         tc.tile_pool(name="psyt", bufs=1, space="PSUM") as psyt:
        for g in range(n_grp):
            agA = aggp.tile([P, GW * WCOLS], F32R, tag="agA")
            agB = aggp.tile([P, GW * WCOLS], F32R, tag="agB")
            for half in range(2):
                call = g * 2 + half
                idxt = idxp.tile([P, IW], I16, tag="ix")
                nc.sync.dma_start(idxt[:], T["t_i1"].ap()[call, :, :])
                st = sp.tile([P, GW, WCOLS], BF16, tag="s")
                nc.sync.dma_start(st[:], T["t_s1"].ap()[call, :, :, :])
                mts = []
                for s in range(NI // 1024):
                    mt = msgs_pool.tile([P, 8, 256], BF16, tag=f"m1_{s}")
                    nc.gpsimd.dma_gather(
                        mt[:], x_views[half],
                        idxt[:, 64 * s:64 * (s + 1)], 1024, 1024, 256,
                        single_packet=False, queue_num=s)
                    mts.append(mt)
                pA = ps1.tile([P, GW * WCOLS], F32, tag="pA")
                pB = ps1b.tile([P, GW * WCOLS], F32, tag="pB")
                for j in range(GW):
                    cs = slice(WCOLS * j, WCOLS * (j + 1))
                    mt = mts[j // 8]
                    jj = j % 8
                    nc.tensor.matmul(pA[:, cs], lhsT=mt[:, jj, 0:P],
                                     rhs=st[:, j, :],
                                     start=(j == 0), stop=(j == GW - 1))
                    nc.tensor.matmul(pB[:, cs], lhsT=mt[:, jj, P:256],
                                     rhs=st[:, j, :],
                                     start=(j == 0), stop=(j == GW - 1))
                if half == 0:
                    nc.scalar.activation(agA[:], pA[:], Copy)
                    nc.vector.tensor_copy(agB[:], pB[:])
                else:
                    nc.vector.tensor_tensor(agA[:], agA[:], pA[:],
                                            op=mybir.AluOpType.add)
                    nc.vector.tensor_tensor(agB[:], agB[:], pB[:],
                                            op=mybir.AluOpType.add)
            cols = slice(g * GW * WCOLS, (g + 1) * GW * WCOLS)
            for m in range(2):
                pz = psz.tile([P, GW * WCOLS], F32, tag="pz")
                nc.tensor.matmul(pz[:], lhsT=w1Ar[:, m * P:(m + 1) * P],
                                 rhs=agA[:], start=True, stop=False)
                nc.tensor.matmul(pz[:], lhsT=w1Br[:, m * P:(m + 1) * P],
                                 rhs=agB[:], start=False, stop=True)
                h1m = h1A if m == 0 else h1B
                b1m = b1A if m == 0 else b1B
                nc.scalar.activation(h1m[:, cols], pz[:], Relu,
                                     bias=b1m[:, 0:1])
            # yab chunk for this group: yT[4, 512] = abA.T@h1A + abB.T@h1B
            py = psy.tile([4, GW * WCOLS], F32, tag="py")
            nc.tensor.matmul(py[:], lhsT=abA[:], rhs=h1A[:, cols],
                             start=True, stop=False)
            nc.tensor.matmul(py[:], lhsT=abB[:], rhs=h1B[:, cols],
                             start=False, stop=True)
            ysb = aggp.tile([4, GW * WCOLS], F32, tag="ysb")
            nc.scalar.activation(ysb[:], py[:], Copy)
            for tt in range(GW * WCOLS // P):
                pyt = psyt.tile([P, 4], F32, tag="pyt")
                nc.tensor.transpose(pyt[:], ysb[:, tt * P:(tt + 1) * P], id4[:])
                nc.vector.tensor_copy(
                    ystage[:, g * (GW * WCOLS // P) + tt, :], pyt[:])
    nc.sync.dma_start(T["t_yab"].ap().rearrange("p (t c) -> p t c", c=4),
                      ystage[:])
    h1pool_cm.__exit__(None, None, None)

    # ---------------- AllGather yab + pad-spray ----------------
    nc.gpsimd.collective_compute(
        "AllGather", mybir.AluOpType.bypass,
        replica_groups=[list(range(n_cores))],
        ins=[T["t_yab"].ap().opt()],
        outs=[T["t_uvc"].ap().opt()],
    )
    uvc_rows = T["t_uvc"].ap().rearrange("a (b c) -> (a b) c", c=4)
    nch = dims["n_cores"] // 2
    uvp_t = [T["t_uvp0"], T["t_uvp1"]]
    # spray each half-table in 3 chunks across sync/scalar/gpsimd (6 streams)
    spray_eng = [nc.sync, nc.scalar, nc.gpsimd]
    for t in range(2):
        nrow = nch * COLS
        cut1 = (nrow // 3) // P * P
        cut2 = (2 * nrow // 3) // P * P
        for hh, (r0, r1) in enumerate(((0, cut1), (cut1, cut2), (cut2, nrow))):
            spray_eng[hh].dma_start(
                uvp_t[t].ap()[r0:r1, 0:4],
                uvc_rows[t * nrow + r0:t * nrow + r1, :])
    tuv_views = [uvp_t[t].ap()[:, :] for t in range(2)]

    # ---------------- layer 2 ----------------
    with tc.tile_pool(name="m2", bufs=4) as m2pool, \
         tc.tile_pool(name="idxp2", bufs=4) as idxp2, \
         tc.tile_pool(name="sp2", bufs=4) as sp2, \
         tc.tile_pool(name="ps2", bufs=2, space="PSUM") as ps2:
        for g in range(n_grp):
            puv = ps2.tile([4, GW * WCOLS], F32, tag="puv")
            for half in range(2):
                call = g * 2 + half
                idxt = idxp2.tile([P, IW], I16, tag="ix2")
                nc.sync.dma_start(idxt[:], T["t_i2"].ap()[call, :, :])
                st = sp2.tile([P, GW, WCOLS], BF16, tag="s2")
                nc.sync.dma_start(st[:], T["t_s1"].ap()[call, :, :, :])
                mt2s = []
                for s in range(NI // 1024):
                    mt2 = m2pool.tile([P, 8, P], BF16, tag=f"m2_{s}")
                    nc.gpsimd.dma_gather(
                        mt2[:], tuv_views[half],
                        idxt[:, 64 * s:64 * (s + 1)], 1024, 1024, P,
                        single_packet=False, queue_num=s)
                    mt2s.append(mt2)
                for j in range(GW):
                    cs = slice(WCOLS * j, WCOLS * (j + 1))
                    nc.tensor.matmul(puv[:, cs], lhsT=mt2s[j // 8][:, j % 8, 0:4],
                                     rhs=st[:, j, :],
                                     start=(half == 0 and j == 0),
                                     stop=(half == 1 and j == GW - 1))
            nc.vector.tensor_tensor(
                uvT[:, g * GW * WCOLS:(g + 1) * GW * WCOLS], puv[:],
                cuv[:, 0:1].to_broadcast([4, GW * WCOLS]),
                op=mybir.AluOpType.add)

    # ---------------- transpose uvT -> node-major, build u/v tables -------
    with tc.tile_pool(name="pst", bufs=2, space="PSUM") as pst:
        for t in range(NT):
            ptp = pst.tile([P, 4], F32, tag="ptp")
            nc.tensor.transpose(ptp[:], uvT[:, t * P:(t + 1) * P], id4[:])
            nc.vector.tensor_copy(uvn[:, t, :], ptp[:])
    upad_rows = T["t_upad"].ap()[:, 0:2].rearrange("(p t) c -> p t c", p=P)
    vpad_rows = T["t_vpad"].ap()[:, 0:2].rearrange("(p t) c -> p t c", p=P)
    nc.sync.dma_start(upad_rows, uvn[:, :, 0:2])
    nc.sync.dma_start(vpad_rows, uvn[:, :, 2:4])

    # ---------------- query head ----------------
    qp = ctx.enter_context(tc.tile_pool(name="qp", bufs=1))
    ug = qp.tile([P, QJ, 64], F32)
    for s in range(QTOT // 1024):
        nc.gpsimd.dma_gather(
            ug[:, 8 * s:8 * (s + 1), :], T["t_upad"].ap()[:, :],
            qu[:, 64 * s:64 * (s + 1)], 1024, 1024, 64,
            single_packet=False, queue_num=s % 4)
    us = qp.tile([P, QJ, 2], F32)
    nc.vector.tensor_copy(us[:], ug[:, :, 0:2])
    a2i_v = T["t_a2i"].ap().rearrange("(j p) c -> p j c", p=P)
    nc.sync.dma_start(a2i_v, us[:])
    vg = qp.tile([P, QJ, 64], F32)
    for s in range(QTOT // 1024):
        nc.gpsimd.dma_gather(
            vg[:, 8 * s:8 * (s + 1), :], T["t_vpad"].ap()[:, :],
            qv[:, 64 * s:64 * (s + 1)], 1024, 1024, 64,
            single_packet=False, queue_num=s % 4)
    nc.gpsimd.collective_compute(
        "AllToAll", mybir.AluOpType.bypass,
        replica_groups=[list(range(n_cores))],
        ins=[T["t_a2i"].ap().opt()],
        outs=[T["t_a2o"].ap().opt()],
    )
    ut2 = qp.tile([P, QJ, 2], F32)
    a2o_v = T["t_a2o"].ap().rearrange("(j p) c -> p j c", p=P)
    nc.sync.dma_start(ut2[:], a2o_v)

    lg = qp.tile([P, QJ, 2], F32)
    nc.vector.tensor_tensor(lg[:], ut2[:], vg[:, :, 0:2],
                            op=mybir.AluOpType.add)
    mx = qp.tile([P, QJ, 1], F32)
    nc.vector.reduce_max(mx[:], lg[:], axis=mybir.AxisListType.X)
    tt = qp.tile([P, QJ, 2], F32)
    nc.vector.tensor_tensor(tt[:], lg[:], mx[:].to_broadcast([P, QJ, 2]),
                            op=mybir.AluOpType.subtract)
    ex = qp.tile([P, QJ, 2], F32)
    nc.scalar.activation(ex[:], tt[:], Exp)
    sm = qp.tile([P, QJ, 1], F32)
    nc.vector.reduce_sum(sm[:], ex[:], axis=mybir.AxisListType.X)
    ls = qp.tile([P, QJ, 1], F32)
    nc.scalar.activation(ls[:], sm[:], Ln)
    oo = qp.tile([P, QJ, 2], F32)
    nc.vector.tensor_tensor(oo[:], tt[:], ls[:].to_broadcast([P, QJ, 2]),
                            op=mybir.AluOpType.subtract)
    nc.sync.dma_start(T["t_out"].ap()[:, :, :], oo[:])


# ----------------------------------------------------------------------------
# numpy reference (mirrors reference.py math in f32)
# ----------------------------------------------------------------------------

def numpy_reference(inputs):
    x = np.asarray(inputs["x"], np.float32)
    ei = np.asarray(inputs["edge_index"], np.int64)
    qe = np.asarray(inputs["query_edges"], np.int64)
    w = np.asarray(inputs["edge_weight"], np.float32)
    W1, b1 = np.asarray(inputs["W1"], np.float32), np.asarray(inputs["b1"], np.float32)
    W2, b2 = np.asarray(inputs["W2"], np.float32), np.asarray(inputs["b2"], np.float32)
    Wl, bl = np.asarray(inputs["Wl"], np.float32), np.asarray(inputs["bl"], np.float32)
    N = x.shape[0]
    src, dst = ei[0], ei[1]

    def conv(h, W, b):
        z = h @ W
        msg = z[src] * w[:, None]
        agg = np.zeros((N, z.shape[1]), np.float32)
        np.add.at(agg, dst, msg)
        return agg + b

    h1 = np.maximum(conv(x, W1, b1), 0.0)
    h2 = conv(h1, W2, b2)
    q = np.concatenate([h2[qe[:, 0]], h2[qe[:, 1]]], axis=1)
    logits = q @ Wl + bl
    m = logits.max(axis=1, keepdims=True)
    e = np.exp(logits - m)
    return logits - m - np.log(e.sum(axis=1, keepdims=True))

# ----------------------------------------------------------------------------
# Entry point: full inputs in, full output out
# ----------------------------------------------------------------------------

LAST_RESULTS = None


def kernel(**inputs):
    """Takes the FULL (unsharded) inputs of nn_DiGCN_link_prediction and
    returns the full [N_QUERY, 2] float32 log-softmax output.

    Shards nodes/edges/queries across 8 NeuronCores internally, runs one
    SPMD Bass kernel (per-core data, identical graph), and reassembles.
    """
    global LAST_RESULTS
    import os
    from concourse.bass_utils import run_bass_kernel_spmd

    n_cores = 8
    dims, in_maps, meta = plan(inputs, n_cores=n_cores)
    nc = build_nc(dims)
    res = run_bass_kernel_spmd(
        nc, in_maps, core_ids=list(range(n_cores)),
        trace=bool(int(os.environ.get("GNN_TRACE", "0"))),
        stitch_traces=False,
    )
    LAST_RESULTS = res
    return unshard(res.results, meta)

